# revision 16
# baseline (speedup 1.0000x reference)
"""Multi-head graph attention (GAT-style message passing) on 8 Trainium2 cores.

Math (per head i, diag transform):
    h        = x * w[i]                      # [N, d]
    p_src    = h @ a[:d],  p_dst = h @ a[d:] # [N]
    s_e      = p_src[src_e] + p_dst[dst_e]   # per edge
    e_e      = exp(-leaky_relu(s_e, 0.2))
    out[i,n] = (sum_{e: src=n} e_e * h[dst_e]) / (sum_{e: src=n} e_e)

Key identity: w[i] (a per-channel diagonal) commutes with the segment sum, so
    out[i,n] = w[i] * (sum e_e * x[dst_e]) / rowsum_n
and the expensive gather of x[dst] is shared by all 4 heads.

Strategy:
  - Host: fold (w, attn) -> A [d, 2H]; sort edges by src; partition nodes
    equally across 8 cores; pack each core's edges into 128-edge tiles that
    never split a node and span <= 16 nodes; groups of 32 tiles.
  - Launch 1 (tiny): P = x @ A computed distributed ([N,8] fp32), host concats.
  - Launch 2 (main, per core): indirect-DMA gather x16[dst] (fp16 rows) and
    P[src], P[dst]; scores -> e on ACT; one-hot matrices on DVE from
    host-provided local offsets; TensorE does the segment sum into PSUM
    (128-edge x 16-node windows, statically placed); rowsums via e-stationary
    matmuls; reciprocal + scale; output written [c-major], host transposes.
"""

import os

import numpy as np

from concourse import bacc, bass, mybir
import concourse.tile as tile
from concourse.bass import IndirectOffsetOnAxis
from concourse.bass_utils import run_bass_kernel_spmd

# test.py sets GAT_TRACE=1 to profile; results of the last kernel() call are
# stashed here so the harness can report HW time.
LAST_RESULTS = []

F32 = mybir.dt.float32
F16 = mybir.dt.float16
I32 = mybir.dt.int32

N_CORES = 8
TILE_E = 128      # edges per tile (partition dim)
W = 16            # max node span of a tile (one-hot width)
TPG = 32          # tiles per group (=> 512 node-slots per group, one PSUM bank row)
EPG = TILE_E * TPG  # edges per group


# --------------------------------------------------------------------------
# host-side layout preprocessing
# --------------------------------------------------------------------------

def _pack_core(src, dst, n_lo, n_hi):
    """Pack one core's (sorted-by-src) edges into tiles.

    Returns (tiles, spans) where tiles is a list of (base_node, edge_idx_array)
    and spans[t] = number of nodes covered by tile t.
    Guarantees: a node's edges are never split across tiles; span <= W;
    <= TILE_E edges per tile.
    """
    # edge index range for this core
    lo = np.searchsorted(src, n_lo, side="left")
    hi = np.searchsorted(src, n_hi, side="left")
    s = src[lo:hi]
    # per-node edge counts within [n_lo, n_hi)
    counts = np.bincount(s - n_lo, minlength=n_hi - n_lo)
    assert counts.max() <= TILE_E, "node degree exceeds one tile"
    starts = lo + np.concatenate([[0], np.cumsum(counts)[:-1]])

    tiles = []
    spans = []
    cur_edges = 0
    cur_base = None
    cur_start = None
    cur_nodes = 0
    for ni in range(n_hi - n_lo):
        c = int(counts[ni])
        node = n_lo + ni
        if cur_base is None:
            cur_base, cur_start, cur_edges, cur_nodes = node, int(starts[ni]), c, 1
            continue
        if cur_edges + c > TILE_E or (node - cur_base) >= W:
            tiles.append((cur_base, cur_start, cur_edges))
            spans.append(cur_nodes)
            cur_base, cur_start, cur_edges, cur_nodes = node, int(starts[ni]), c, 1
        else:
            cur_edges += c
            cur_nodes = node - cur_base + 1
    if cur_base is not None:
        tiles.append((cur_base, cur_start, cur_edges))
        spans.append(cur_nodes)
    return tiles, spans


def _prep_edges(src, dst, n_nodes):
    """Sort by src, partition nodes across cores, build per-core tile arrays.

    Returns dict with per-core arrays (lists of length N_CORES) and G.
    """
    order = np.argsort(src, kind="stable")
    src_s = src[order].astype(np.int32)
    dst_s = dst[order].astype(np.int32)

    npc = n_nodes // N_CORES
    per_core = []
    for c in range(N_CORES):
        n_lo, n_hi = c * npc, (c + 1) * npc if c < N_CORES - 1 else n_nodes
        tiles, spans = _pack_core(src_s, dst_s, n_lo, n_hi)
        per_core.append((tiles, spans))

    G = max((len(t[0]) + TPG - 1) // TPG for t in per_core)

    dsti, srci, loc, colmap = [], [], [], []
    for c in range(N_CORES):
        tiles, spans = per_core[c]
        nt = G * TPG
        d_idx = np.zeros((nt, TILE_E), dtype=np.int32)
        s_idx = np.zeros((nt, TILE_E), dtype=np.int32)
        l_arr = np.full((nt, TILE_E), -1.0, dtype=np.float16)
        cmap = np.full((nt, W), -1, dtype=np.int64)
        for t, (base, estart, ecnt) in enumerate(tiles):
            d_idx[t, :ecnt] = dst_s[estart:estart + ecnt]
            s_idx[t, :ecnt] = src_s[estart:estart + ecnt]
            l_arr[t, :ecnt] = (src_s[estart:estart + ecnt] - base).astype(np.float16)
            cmap[t, :spans[t]] = np.arange(base, base + spans[t])
        # reshape to [G, 128, TPG]: tile t of group g at [:, t], edge p on partition p
        d4 = d_idx.reshape(G, TPG, TILE_E).transpose(0, 2, 1).copy()
        s4 = s_idx.reshape(G, TPG, TILE_E).transpose(0, 2, 1).copy()
        l4 = l_arr.reshape(G, TPG, TILE_E).transpose(0, 2, 1).copy()
        dsti.append(d4)
        srci.append(s4)
        loc.append(l4)
        colmap.append(cmap.reshape(G * TPG * W))
    return dict(dsti=dsti, srci=srci, loc=loc, colmap=colmap, G=G)


# --------------------------------------------------------------------------
# launch 1: P = x @ A   (distributed over node slabs)
# --------------------------------------------------------------------------

def _build_l1(nt):
    """xt: [128, nt*128] f32 (= x-slab transposed), amat: [128, 8] f32
    -> pout: [nt*128, 8] f32"""
    nc = bacc.Bacc(None)
    xt = nc.declare_dram_parameter("xt", [128, nt * 128], F32, isOutput=False)
    amat = nc.declare_dram_parameter("amat", [128, 8], F32, isOutput=False)
    pout = nc.declare_dram_parameter("pout", [nt * 128, 8], F32, isOutput=True)

    with tile.TileContext(nc) as tc:
        with (
            tc.tile_pool(name="sb", bufs=3) as sb,
            tc.tile_pool(name="cst", bufs=1) as cst,
            tc.tile_pool(name="ps", bufs=2, space="PSUM") as ps,
        ):
            a_sb = cst.tile([128, 8], F32)
            nc.sync.dma_start(out=a_sb[:], in_=amat[:, :])
            # walrus only allows one sync wait on a Matmult(LDW); this tiny
            # carrier matmul absorbs the a_sb DMA wait so the real matmuls
            # each carry a single xt-tile wait.
            dummy_ps = ps.tile([1, 1], F32, tag="dummy")
            nc.tensor.matmul(out=dummy_ps[:], lhsT=a_sb[:1, :1], rhs=a_sb[:1, :1],
                             start=True, stop=True)
            for t in range(nt):
                xt_sb = sb.tile([128, 128], F32, tag="xt")
                nc.sync.dma_start(out=xt_sb[:], in_=xt[:, t * 128:(t + 1) * 128])
                p_ps = ps.tile([128, 8], F32)
                nc.tensor.matmul(out=p_ps[:], lhsT=xt_sb[:], rhs=a_sb[:],
                                 start=True, stop=True)
                p_sb = sb.tile([128, 8], F32, tag="p")
                nc.vector.tensor_copy(out=p_sb[:], in_=p_ps[:])
                nc.sync.dma_start(out=pout[t * 128:(t + 1) * 128, :], in_=p_sb[:])
    nc.compile()
    return nc


# --------------------------------------------------------------------------
# launch 2: the main edge-parallel kernel
# --------------------------------------------------------------------------

def _build_l2(n_nodes, G):
    nc = bacc.Bacc(None)
    x16 = nc.declare_dram_parameter("x16", [n_nodes, 128], F16, isOutput=False)
    ptab = nc.declare_dram_parameter("ptab", [n_nodes, 8], F32, isOutput=False)
    dsti = nc.declare_dram_parameter("dsti", [G, 128, TPG], I32, isOutput=False)
    srci = nc.declare_dram_parameter("srci", [G, 128, TPG], I32, isOutput=False)
    locd = nc.declare_dram_parameter("locd", [G, 128, TPG], F16, isOutput=False)
    iotac = nc.declare_dram_parameter("iotac", [128, W], F16, isOutput=False)
    selc = nc.declare_dram_parameter("selc", [4, 512], F16, isOutput=False)
    wcol = nc.declare_dram_parameter("wcol", [128, 4], F32, isOutput=False)
    out = nc.declare_dram_parameter("out", [4, G, 128, TPG * W], F32, isOutput=True)

    with tile.TileContext(nc) as tc:
        with (
            tc.tile_pool(name="cst", bufs=1) as cst,
            tc.tile_pool(name="idx", bufs=3) as idxp,
            tc.tile_pool(name="gat", bufs=2) as gat,
            tc.tile_pool(name="mm", bufs=2) as mm,
            tc.tile_pool(name="epi", bufs=2) as epi,
            tc.tile_pool(name="outp", bufs=4) as outp,
            tc.tile_pool(name="ps", bufs=1, space="PSUM") as ps,
            tc.tile_pool(name="psb", bufs=2, space="PSUM") as psb,
        ):
            iota_sb = cst.tile([128, W], F16)
            nc.sync.dma_start(out=iota_sb[:], in_=iotac[:, :])
            sel_sb = cst.tile([4, 512], F16)
            nc.sync.dma_start(out=sel_sb[:], in_=selc[:, :])
            w_sb = cst.tile([128, 4], F32)
            nc.sync.dma_start(out=w_sb[:], in_=wcol[:, :])

            for g in range(G):
                # ---- per-group metadata loads
                di = idxp.tile([128, TPG], I32, tag="di")
                si = idxp.tile([128, TPG], I32, tag="si")
                lo = idxp.tile([128, TPG], F16, tag="lo")
                nc.sync.dma_start(out=di[:], in_=dsti[g, :, :])
                nc.sync.dma_start(out=si[:], in_=srci[g, :, :])
                nc.sync.dma_start(out=lo[:], in_=locd[g, :, :])

                # ---- gathers (HW only honors one offset per partition, so
                # issue per-tile [128,1] indirect DMAs)
                xg = gat.tile([128, TPG, 128], F16, tag="xg")
                psg = gat.tile([128, TPG, 8], F32, tag="psg")
                pdg = gat.tile([128, TPG, 8], F32, tag="pdg")
                for t in range(TPG):
                    nc.gpsimd.indirect_dma_start(
                        out=xg[:, t, :], out_offset=None, in_=x16[:, :],
                        in_offset=IndirectOffsetOnAxis(ap=di[:, t:t + 1], axis=0))
                    nc.gpsimd.indirect_dma_start(
                        out=psg[:, t, :], out_offset=None, in_=ptab[:, :],
                        in_offset=IndirectOffsetOnAxis(ap=si[:, t:t + 1], axis=0))
                    nc.gpsimd.indirect_dma_start(
                        out=pdg[:, t, :], out_offset=None, in_=ptab[:, :],
                        in_offset=IndirectOffsetOnAxis(ap=di[:, t:t + 1], axis=0))

                # ---- scores: e = exp(-leaky_relu(p_src[src] + p_dst[dst]))
                s32 = mm.tile([128, TPG, 4], F32, tag="s32")
                nc.vector.tensor_tensor(out=s32[:], in0=psg[:, :, 0:4],
                                        in1=pdg[:, :, 4:8],
                                        op=mybir.AluOpType.add)
                y32 = mm.tile([128, TPG, 4], F32, tag="y32")
                # leaky_relu(s) = max(0.2*s, s)
                nc.vector.scalar_tensor_tensor(
                    out=y32[:], in0=s32[:], scalar=0.2, in1=s32[:],
                    op0=mybir.AluOpType.mult, op1=mybir.AluOpType.max)
                e16 = mm.tile([128, TPG, 4], F16, tag="e16")
                nc.scalar.activation(out=e16[:], in_=y32[:],
                                     func=mybir.ActivationFunctionType.Exp,
                                     scale=-1.0)

                # ---- one-hot matrices
                m0 = mm.tile([128, TPG, W], F16, tag="m0")
                nc.vector.tensor_tensor(
                    out=m0[:],
                    in0=lo[:, :, None].broadcast_to([128, TPG, W]),
                    in1=iota_sb[:, None, :].broadcast_to([128, TPG, W]),
                    op=mybir.AluOpType.is_equal)
                mall = mm.tile([128, TPG, 4, W], F16, tag="mall")
                nc.vector.tensor_tensor(
                    out=mall[:],
                    in0=m0[:, :, None, :].broadcast_to([128, TPG, 4, W]),
                    in1=e16[:, :, :, None].broadcast_to([128, TPG, 4, W]),
                    op=mybir.AluOpType.mult)

                # ---- segment sums on TensorE
                agg = ps.tile([128, TPG * 4 * W], F32, tag="agg")
                rs = ps.tile([4, TPG * W], F32, tag="rs")
                for t in range(TPG):
                    nc.tensor.matmul(
                        out=agg[:, t * 4 * W:(t + 1) * 4 * W],
                        lhsT=xg[:, t, :], rhs=mall[:, t, :, :],
                        start=True, stop=True)
                    nc.tensor.matmul(
                        out=rs[:, t * W:(t + 1) * W],
                        lhsT=e16[:, t, :], rhs=m0[:, t, :],
                        start=True, stop=True)

                # ---- epilogue: out = w ⊙ agg / rowsum
                # clamp pad-column zeros so reciprocal stays finite (real
                # rowsums are >= exp(-|s|max) >> 3e-5)
                rsc = epi.tile([4, TPG * W], F32, tag="rsc")
                nc.vector.tensor_scalar(out=rsc[:], in0=rs[:], scalar1=3e-5,
                                        scalar2=None, op0=mybir.AluOpType.max)
                rsi32 = epi.tile([4, TPG * W], F32, tag="rsi32")
                nc.vector.reciprocal(out=rsi32[:], in_=rsc[:])
                rsi16 = epi.tile([4, TPG * W], F16, tag="rsi16")
                nc.vector.tensor_copy(out=rsi16[:], in_=rsi32[:])
                agg4 = agg[:].rearrange("p (t h w) -> p t h w", t=TPG, h=4, w=W)
                for i in range(4):
                    bc = psb.tile([128, TPG * W], F32, tag="bc")
                    nc.tensor.matmul(out=bc[:], lhsT=sel_sb[:, i * 128:(i + 1) * 128],
                                     rhs=rsi16[:], start=True, stop=True)
                    rinv = epi.tile([128, TPG * W], F32, tag="rinv")
                    nc.scalar.activation(out=rinv[:], in_=bc[:],
                                         func=mybir.ActivationFunctionType.Copy)
                    oh = outp.tile([128, TPG * W], F32, tag="oh")
                    oh4 = oh[:].rearrange("p (t w) -> p t w", t=TPG, w=W)
                    rinv4 = rinv[:].rearrange("p (t w) -> p t w", t=TPG, w=W)
                    nc.vector.scalar_tensor_tensor(
                        out=oh4, in0=agg4[:, :, i, :],
                        scalar=w_sb[:, i:i + 1],
                        in1=rinv4,
                        op0=mybir.AluOpType.mult, op1=mybir.AluOpType.mult)
                    nc.sync.dma_start(out=out[i, g, :, :], in_=oh[:])
    nc.compile()
    return nc


# --------------------------------------------------------------------------
# entry point
# --------------------------------------------------------------------------

def kernel(x, w, attn, edge):
    x = np.asarray(x, dtype=np.float32)
    w = np.asarray(w, dtype=np.float32)
    attn = np.asarray(attn, dtype=np.float32)
    edge = np.asarray(edge)

    n_nodes, d = x.shape
    n_heads = w.shape[0]
    assert d == 128 and n_heads == 4

    src = edge[0].astype(np.int64)
    dst = edge[1].astype(np.int64)

    # fold parameters: A[:, i] = w_i * a_src_i ; A[:, 4+i] = w_i * a_dst_i
    amat = np.zeros((128, 8), dtype=np.float32)
    for i in range(n_heads):
        amat[:, i] = w[i, 0, :] * attn[i, :d, 0]
        amat[:, 4 + i] = w[i, 0, :] * attn[i, d:, 0]

    # ---------------- launch 1: P = x @ A (node slabs)
    npc = n_nodes // N_CORES
    nt = (npc + 127) // 128
    nc1 = _build_l1(nt)
    in_maps1 = []
    for c in range(N_CORES):
        sl = x[c * npc:(c + 1) * npc]
        if sl.shape[0] < nt * 128:
            sl = np.concatenate(
                [sl, np.zeros((nt * 128 - sl.shape[0], d), np.float32)])
        in_maps1.append({"xt": np.ascontiguousarray(sl.T), "amat": amat})
    trace = bool(int(os.environ.get("GAT_TRACE", "0")))
    tkw = dict(trace=True, trace_cores=list(range(N_CORES))) if trace else {}

    def _run(nc, maps):
        try:
            return run_bass_kernel_spmd(nc, maps, list(range(N_CORES)), **tkw)
        except Exception:
            if not tkw:
                raise
            return run_bass_kernel_spmd(nc, maps, list(range(N_CORES)))

    r1 = _run(nc1, in_maps1)
    ptab = np.concatenate(
        [r1.results[c]["pout"][:npc] for c in range(N_CORES)], axis=0)
    ptab = np.ascontiguousarray(ptab[:n_nodes])

    # ---------------- host layout prep
    prep = _prep_edges(src, dst, n_nodes)
    G = prep["G"]

    # ---------------- launch 2
    nc2 = _build_l2(n_nodes, G)
    x16 = x.astype(np.float16)
    iota_c = np.broadcast_to(np.arange(W, dtype=np.float16), (128, W)).copy()
    sel_c = np.zeros((4, 512), dtype=np.float16)
    for i in range(4):
        sel_c[i, i * 128:(i + 1) * 128] = 1.0
    wcol = np.ascontiguousarray(w[:, 0, :].T)  # [128, 4]
    in_maps2 = []
    for c in range(N_CORES):
        in_maps2.append({
            "x16": x16, "ptab": ptab,
            "dsti": prep["dsti"][c], "srci": prep["srci"][c],
            "locd": prep["loc"][c],
            "iotac": iota_c, "selc": sel_c, "wcol": wcol,
        })
    r2 = _run(nc2, in_maps2)
    LAST_RESULTS.clear()
    LAST_RESULTS.extend([r1, r2])

    # ---------------- unshard: scatter tile-local columns to node rows
    out_full = np.zeros((n_heads, n_nodes, d), dtype=np.float32)
    for c in range(N_CORES):
        slab = r2.results[c]["out"]  # [4, G, 128, TPG*W]
        cm = prep["colmap"][c]       # [G*TPG*W] -> node or -1
        arr = slab.transpose(0, 1, 3, 2).reshape(n_heads, G * TPG * W, d)
        valid = cm >= 0
        out_full[:, cm[valid], :] = arr[:, valid, :]
    return out_full


if __name__ == "__main__":
    # smoke test with the real shapes is done via test.py
    pass


# revision 21
# speedup vs baseline: 1.4769x; 1.4769x over previous
"""Multi-head graph attention (GAT-style message passing) on 8 Trainium2 cores.

Math (per head i, diag transform):
    h        = x * w[i]                      # [N, d]
    p_src    = h @ a[:d],  p_dst = h @ a[d:] # [N]
    s_e      = p_src[src_e] + p_dst[dst_e]   # per edge
    e_e      = exp(-leaky_relu(s_e, 0.2))
    out[i,n] = (sum_{e: src=n} e_e * h[dst_e]) / (sum_{e: src=n} e_e)

Key identity: w[i] (a per-channel diagonal) commutes with the segment sum, so
    out[i,n] = w[i] * (sum e_e * x[dst_e]) / rowsum_n
and the expensive gather of x[dst] is shared by all 4 heads.

Strategy:
  - Host: fold (w, attn) -> A [d, 2H]; sort edges by src; partition nodes
    equally across 8 cores; pack each core's edges into 128-edge tiles that
    never split a node and span <= 16 nodes; groups of 32 tiles.
  - Launch 1 (tiny): P = x @ A computed distributed ([N,8] fp32), host concats.
  - Launch 2 (main, per core): indirect-DMA gather x16[dst] (fp16 rows) and
    P[src], P[dst]; scores -> e on ACT; one-hot matrices on DVE from
    host-provided local offsets; TensorE does the segment sum into PSUM
    (128-edge x 16-node windows, statically placed); rowsums via e-stationary
    matmuls; reciprocal + scale; output written [c-major], host transposes.
"""

import os

import numpy as np

from concourse import bacc, bass, mybir
import concourse.tile as tile
from concourse.bass import IndirectOffsetOnAxis
from concourse.bass_utils import run_bass_kernel_spmd

# test.py sets GAT_TRACE=1 to profile; results of the last kernel() call are
# stashed here so the harness can report HW time.
LAST_RESULTS = []

F32 = mybir.dt.float32
F16 = mybir.dt.float16
I32 = mybir.dt.int32

N_CORES = 8
TILE_E = 128      # edges per tile (partition dim)
W = 16            # max node span of a tile (one-hot width)
TPG = 32          # tiles per group (=> 512 node-slots per group, one PSUM bank row)
EPG = TILE_E * TPG  # edges per group


# --------------------------------------------------------------------------
# host-side layout preprocessing
# --------------------------------------------------------------------------

def _pack_core(src, dst, n_lo, n_hi):
    """Pack one core's (sorted-by-src) edges into tiles.

    Returns (tiles, spans) where tiles is a list of (base_node, edge_idx_array)
    and spans[t] = number of nodes covered by tile t.
    Guarantees: a node's edges are never split across tiles; span <= W;
    <= TILE_E edges per tile.
    """
    # edge index range for this core
    lo = np.searchsorted(src, n_lo, side="left")
    hi = np.searchsorted(src, n_hi, side="left")
    s = src[lo:hi]
    # per-node edge counts within [n_lo, n_hi)
    counts = np.bincount(s - n_lo, minlength=n_hi - n_lo)
    assert counts.max() <= TILE_E, "node degree exceeds one tile"
    starts = lo + np.concatenate([[0], np.cumsum(counts)[:-1]])

    tiles = []
    spans = []
    cur_edges = 0
    cur_base = None
    cur_start = None
    cur_nodes = 0
    for ni in range(n_hi - n_lo):
        c = int(counts[ni])
        node = n_lo + ni
        if cur_base is None:
            cur_base, cur_start, cur_edges, cur_nodes = node, int(starts[ni]), c, 1
            continue
        if cur_edges + c > TILE_E or (node - cur_base) >= W:
            tiles.append((cur_base, cur_start, cur_edges))
            spans.append(cur_nodes)
            cur_base, cur_start, cur_edges, cur_nodes = node, int(starts[ni]), c, 1
        else:
            cur_edges += c
            cur_nodes = node - cur_base + 1
    if cur_base is not None:
        tiles.append((cur_base, cur_start, cur_edges))
        spans.append(cur_nodes)
    return tiles, spans


def _prep_edges(src, dst, n_nodes):
    """Sort by src, partition nodes across cores, build per-core tile arrays.

    Returns dict with per-core arrays (lists of length N_CORES) and G.
    """
    order = np.argsort(src, kind="stable")
    src_s = src[order].astype(np.int32)
    dst_s = dst[order].astype(np.int32)

    npc = n_nodes // N_CORES
    per_core = []
    for c in range(N_CORES):
        n_lo, n_hi = c * npc, (c + 1) * npc if c < N_CORES - 1 else n_nodes
        tiles, spans = _pack_core(src_s, dst_s, n_lo, n_hi)
        per_core.append((tiles, spans))

    G = max((len(t[0]) + TPG - 1) // TPG for t in per_core)

    dsti, srci, loc, colmap = [], [], [], []
    for c in range(N_CORES):
        tiles, spans = per_core[c]
        nt = G * TPG
        d_idx = np.zeros((nt, TILE_E), dtype=np.int32)
        s_idx = np.zeros((nt, TILE_E), dtype=np.int32)
        l_arr = np.full((nt, TILE_E), -1.0, dtype=np.float16)
        cmap = np.full((nt, W), -1, dtype=np.int64)
        for t, (base, estart, ecnt) in enumerate(tiles):
            d_idx[t, :ecnt] = dst_s[estart:estart + ecnt]
            s_idx[t, :ecnt] = src_s[estart:estart + ecnt]
            l_arr[t, :ecnt] = (src_s[estart:estart + ecnt] - base).astype(np.float16)
            cmap[t, :spans[t]] = np.arange(base, base + spans[t])
        # reshape to [G, 128, TPG]: tile t of group g at [:, t], edge p on partition p
        d4 = d_idx.reshape(G, TPG, TILE_E).transpose(0, 2, 1).copy()
        s4 = s_idx.reshape(G, TPG, TILE_E).transpose(0, 2, 1).copy()
        l4 = l_arr.reshape(G, TPG, TILE_E).transpose(0, 2, 1).copy()
        dsti.append(d4)
        srci.append(s4)
        loc.append(l4)
        colmap.append(cmap.reshape(G * TPG * W))
    return dict(dsti=dsti, srci=srci, loc=loc, colmap=colmap, G=G)


# --------------------------------------------------------------------------
# launch 1: P = x @ A   (distributed over node slabs)
# --------------------------------------------------------------------------

def _build_l1(nt):
    """xt: [128, nt*128] f32 (= x-slab transposed), amat: [128, 8] f32
    -> pout: [nt*128, 8] f32"""
    nc = bacc.Bacc(None)
    xt = nc.declare_dram_parameter("xt", [128, nt * 128], F32, isOutput=False)
    amat = nc.declare_dram_parameter("amat", [128, 8], F32, isOutput=False)
    pout = nc.declare_dram_parameter("pout", [nt * 128, 8], F32, isOutput=True)

    with tile.TileContext(nc) as tc:
        with (
            tc.tile_pool(name="sb", bufs=3) as sb,
            tc.tile_pool(name="cst", bufs=1) as cst,
            tc.tile_pool(name="ps", bufs=2, space="PSUM") as ps,
        ):
            a_sb = cst.tile([128, 8], F32)
            nc.sync.dma_start(out=a_sb[:], in_=amat[:, :])
            # walrus only allows one sync wait on a Matmult(LDW); this tiny
            # carrier matmul absorbs the a_sb DMA wait so the real matmuls
            # each carry a single xt-tile wait.
            dummy_ps = ps.tile([1, 1], F32, tag="dummy")
            nc.tensor.matmul(out=dummy_ps[:], lhsT=a_sb[:1, :1], rhs=a_sb[:1, :1],
                             start=True, stop=True)
            for t in range(nt):
                xt_sb = sb.tile([128, 128], F32, tag="xt")
                nc.sync.dma_start(out=xt_sb[:], in_=xt[:, t * 128:(t + 1) * 128])
                p_ps = ps.tile([128, 8], F32)
                nc.tensor.matmul(out=p_ps[:], lhsT=xt_sb[:], rhs=a_sb[:],
                                 start=True, stop=True)
                p_sb = sb.tile([128, 8], F32, tag="p")
                nc.vector.tensor_copy(out=p_sb[:], in_=p_ps[:])
                nc.sync.dma_start(out=pout[t * 128:(t + 1) * 128, :], in_=p_sb[:])
    nc.compile()
    return nc


# --------------------------------------------------------------------------
# launch 2: the main edge-parallel kernel
# --------------------------------------------------------------------------

def _build_l2(n_nodes, G):
    nc = bacc.Bacc(None)
    # t16 row n = [x16[n] (128) | P16[n] (8)] so one gather serves both the
    # feature row and p_dst
    t16 = nc.declare_dram_parameter("t16", [n_nodes, 136], F16, isOutput=False)
    ptab = nc.declare_dram_parameter("ptab", [n_nodes, 8], F32, isOutput=False)
    dsti = nc.declare_dram_parameter("dsti", [G, 128, TPG], I32, isOutput=False)
    srci = nc.declare_dram_parameter("srci", [G, 128, TPG], I32, isOutput=False)
    locd = nc.declare_dram_parameter("locd", [G, 128, TPG], F16, isOutput=False)
    iotac = nc.declare_dram_parameter("iotac", [128, W], F16, isOutput=False)
    selc = nc.declare_dram_parameter("selc", [4, 512], F16, isOutput=False)
    wcol = nc.declare_dram_parameter("wcol", [128, 4], F32, isOutput=False)
    out = nc.declare_dram_parameter("out", [4, G, 128, TPG * W], F32, isOutput=True)

    with tile.TileContext(nc) as tc:
        with (
            tc.tile_pool(name="cst", bufs=1) as cst,
            tc.tile_pool(name="idx", bufs=3) as idxp,
            tc.tile_pool(name="gat", bufs=2) as gat,
            tc.tile_pool(name="mm", bufs=2) as mm,
            tc.tile_pool(name="epi", bufs=2) as epi,
            tc.tile_pool(name="outp", bufs=4) as outp,
            tc.tile_pool(name="ps", bufs=1, space="PSUM") as ps,
            tc.tile_pool(name="psb", bufs=2, space="PSUM") as psb,
        ):
            iota_sb = cst.tile([128, W], F16)
            nc.sync.dma_start(out=iota_sb[:], in_=iotac[:, :])
            sel_sb = cst.tile([4, 512], F16)
            nc.sync.dma_start(out=sel_sb[:], in_=selc[:, :])
            w_sb = cst.tile([128, 4], F32)
            nc.sync.dma_start(out=w_sb[:], in_=wcol[:, :])

            for g in range(G):
                # ---- per-group metadata loads
                di = idxp.tile([128, TPG], I32, tag="di")
                si = idxp.tile([128, TPG], I32, tag="si")
                lo = idxp.tile([128, TPG], F16, tag="lo")
                nc.sync.dma_start(out=di[:], in_=dsti[g, :, :])
                nc.sync.dma_start(out=si[:], in_=srci[g, :, :])
                nc.sync.dma_start(out=lo[:], in_=locd[g, :, :])

                # ---- gathers (HW only honors one offset per partition, so
                # issue per-tile [128,1] indirect DMAs)
                xg = gat.tile([128, TPG, 136], F16, tag="xg")
                psg = gat.tile([128, TPG, 8], F32, tag="psg")
                for t in range(TPG):
                    nc.gpsimd.indirect_dma_start(
                        out=xg[:, t, :], out_offset=None, in_=t16[:, :],
                        in_offset=IndirectOffsetOnAxis(ap=di[:, t:t + 1], axis=0))
                    nc.gpsimd.indirect_dma_start(
                        out=psg[:, t, :], out_offset=None, in_=ptab[:, :],
                        in_offset=IndirectOffsetOnAxis(ap=si[:, t:t + 1], axis=0))

                # ---- scores: e = exp(-leaky_relu(p_src[src] + p_dst[dst]))
                pd32 = mm.tile([128, TPG, 4], F32, tag="pd32")
                nc.vector.tensor_copy(out=pd32[:], in_=xg[:, :, 132:136])
                s32 = mm.tile([128, TPG, 4], F32, tag="s32")
                nc.vector.tensor_tensor(out=s32[:], in0=psg[:, :, 0:4],
                                        in1=pd32[:],
                                        op=mybir.AluOpType.add)
                y32 = mm.tile([128, TPG, 4], F32, tag="y32")
                # leaky_relu(s) = max(0.2*s, s)
                nc.vector.scalar_tensor_tensor(
                    out=y32[:], in0=s32[:], scalar=0.2, in1=s32[:],
                    op0=mybir.AluOpType.mult, op1=mybir.AluOpType.max)
                e16 = mm.tile([128, TPG, 4], F16, tag="e16")
                nc.scalar.activation(out=e16[:], in_=y32[:],
                                     func=mybir.ActivationFunctionType.Exp,
                                     scale=-1.0)

                # ---- one-hot matrices
                m0 = mm.tile([128, TPG, W], F16, tag="m0")
                nc.vector.tensor_tensor(
                    out=m0[:],
                    in0=lo[:, :, None].broadcast_to([128, TPG, W]),
                    in1=iota_sb[:, None, :].broadcast_to([128, TPG, W]),
                    op=mybir.AluOpType.is_equal)
                mall = mm.tile([128, TPG, 4, W], F16, tag="mall")
                nc.vector.tensor_tensor(
                    out=mall[:],
                    in0=m0[:, :, None, :].broadcast_to([128, TPG, 4, W]),
                    in1=e16[:, :, :, None].broadcast_to([128, TPG, 4, W]),
                    op=mybir.AluOpType.mult)

                # ---- segment sums on TensorE
                agg = ps.tile([128, TPG * 4 * W], F32, tag="agg")
                rs = ps.tile([4, TPG * W], F32, tag="rs")
                for t in range(TPG):
                    nc.tensor.matmul(
                        out=agg[:, t * 4 * W:(t + 1) * 4 * W],
                        lhsT=xg[:, t, 0:128], rhs=mall[:, t, :, :],
                        start=True, stop=True)
                    nc.tensor.matmul(
                        out=rs[:, t * W:(t + 1) * W],
                        lhsT=e16[:, t, :], rhs=m0[:, t, :],
                        start=True, stop=True)

                # ---- epilogue: out = w ⊙ agg / rowsum
                # clamp pad-column zeros so reciprocal stays finite (real
                # rowsums are >= exp(-|s|max) >> 3e-5)
                rsc = epi.tile([4, TPG * W], F32, tag="rsc")
                nc.vector.tensor_scalar(out=rsc[:], in0=rs[:], scalar1=3e-5,
                                        scalar2=None, op0=mybir.AluOpType.max)
                rsi32 = epi.tile([4, TPG * W], F32, tag="rsi32")
                nc.vector.reciprocal(out=rsi32[:], in_=rsc[:])
                rsi16 = epi.tile([4, TPG * W], F16, tag="rsi16")
                nc.vector.tensor_copy(out=rsi16[:], in_=rsi32[:])
                agg4 = agg[:].rearrange("p (t h w) -> p t h w", t=TPG, h=4, w=W)
                for i in range(4):
                    bc = psb.tile([128, TPG * W], F32, tag="bc")
                    nc.tensor.matmul(out=bc[:], lhsT=sel_sb[:, i * 128:(i + 1) * 128],
                                     rhs=rsi16[:], start=True, stop=True)
                    rinv = epi.tile([128, TPG * W], F32, tag="rinv")
                    nc.scalar.activation(out=rinv[:], in_=bc[:],
                                         func=mybir.ActivationFunctionType.Copy)
                    oh = outp.tile([128, TPG * W], F32, tag="oh")
                    oh4 = oh[:].rearrange("p (t w) -> p t w", t=TPG, w=W)
                    rinv4 = rinv[:].rearrange("p (t w) -> p t w", t=TPG, w=W)
                    nc.vector.scalar_tensor_tensor(
                        out=oh4, in0=agg4[:, :, i, :],
                        scalar=w_sb[:, i:i + 1],
                        in1=rinv4,
                        op0=mybir.AluOpType.mult, op1=mybir.AluOpType.mult)
                    nc.sync.dma_start(out=out[i, g, :, :], in_=oh[:])
    nc.compile()
    return nc


# --------------------------------------------------------------------------
# entry point
# --------------------------------------------------------------------------

def kernel(x, w, attn, edge):
    x = np.asarray(x, dtype=np.float32)
    w = np.asarray(w, dtype=np.float32)
    attn = np.asarray(attn, dtype=np.float32)
    edge = np.asarray(edge)

    n_nodes, d = x.shape
    n_heads = w.shape[0]
    assert d == 128 and n_heads == 4

    src = edge[0].astype(np.int64)
    dst = edge[1].astype(np.int64)

    # fold parameters: A[:, i] = w_i * a_src_i ; A[:, 4+i] = w_i * a_dst_i
    amat = np.zeros((128, 8), dtype=np.float32)
    for i in range(n_heads):
        amat[:, i] = w[i, 0, :] * attn[i, :d, 0]
        amat[:, 4 + i] = w[i, 0, :] * attn[i, d:, 0]

    # ---------------- launch 1: P = x @ A (node slabs)
    npc = n_nodes // N_CORES
    nt = (npc + 127) // 128
    nc1 = _build_l1(nt)
    in_maps1 = []
    for c in range(N_CORES):
        sl = x[c * npc:(c + 1) * npc]
        if sl.shape[0] < nt * 128:
            sl = np.concatenate(
                [sl, np.zeros((nt * 128 - sl.shape[0], d), np.float32)])
        in_maps1.append({"xt": np.ascontiguousarray(sl.T), "amat": amat})
    trace = bool(int(os.environ.get("GAT_TRACE", "0")))
    tkw = dict(trace=True, trace_cores=list(range(N_CORES))) if trace else {}

    def _run(nc, maps):
        try:
            return run_bass_kernel_spmd(nc, maps, list(range(N_CORES)), **tkw)
        except Exception:
            if not tkw:
                raise
            return run_bass_kernel_spmd(nc, maps, list(range(N_CORES)))

    r1 = _run(nc1, in_maps1)
    ptab = np.concatenate(
        [r1.results[c]["pout"][:npc] for c in range(N_CORES)], axis=0)
    ptab = np.ascontiguousarray(ptab[:n_nodes])

    # ---------------- host layout prep
    prep = _prep_edges(src, dst, n_nodes)
    G = prep["G"]

    # ---------------- launch 2
    nc2 = _build_l2(n_nodes, G)
    x16 = x.astype(np.float16)
    t16 = np.concatenate([x16, ptab.astype(np.float16)], axis=1)
    iota_c = np.broadcast_to(np.arange(W, dtype=np.float16), (128, W)).copy()
    sel_c = np.zeros((4, 512), dtype=np.float16)
    for i in range(4):
        sel_c[i, i * 128:(i + 1) * 128] = 1.0
    wcol = np.ascontiguousarray(w[:, 0, :].T)  # [128, 4]
    in_maps2 = []
    for c in range(N_CORES):
        in_maps2.append({
            "t16": t16, "ptab": ptab,
            "dsti": prep["dsti"][c], "srci": prep["srci"][c],
            "locd": prep["loc"][c],
            "iotac": iota_c, "selc": sel_c, "wcol": wcol,
        })
    r2 = _run(nc2, in_maps2)
    LAST_RESULTS.clear()
    LAST_RESULTS.extend([r1, r2])

    # ---------------- unshard: scatter tile-local columns to node rows
    out_full = np.zeros((n_heads, n_nodes, d), dtype=np.float32)
    for c in range(N_CORES):
        slab = r2.results[c]["out"]  # [4, G, 128, TPG*W]
        cm = prep["colmap"][c]       # [G*TPG*W] -> node or -1
        arr = slab.transpose(0, 1, 3, 2).reshape(n_heads, G * TPG * W, d)
        valid = cm >= 0
        out_full[:, cm[valid], :] = arr[:, valid, :]
    return out_full


if __name__ == "__main__":
    # smoke test with the real shapes is done via test.py
    pass
